# revision 1
# baseline (speedup 1.0000x reference)
"""HaarConv2D (depthwise 2x2 stride-2 Haar transform) on 8 Trainium2 cores.

Input  x: [16, 64, 512, 512] f32
Output (low_pass, detail): each [16, 64, 256, 256] f32
  low  = 0.5*(a+b+c+d),  det = 0.5*(a-b-c+d)  over each 2x2 block
       = (ph + qh, ph - qh)  with ph = 0.5*(a+d), qh = 0.5*(b+c)

Sharding: pure data parallel over batch — core i handles batches [2i, 2i+1].
Per-core layout: partition p = (b_local*64 + channel) plane (128 planes),
free dim = rows of that plane's 512x512 image. Per iteration we load 2R
input rows per plane (contiguous in HBM), compute R output rows, store.

Engine split (memory-bound kernel; DMA is the roofline):
  DVE    p = a+d, q = b+c      (strided-AP tensor_tensor, fp32 1x)
  ACT    ph = 0.5p, qh = 0.5q  (activation Copy with scale)
  GpSimd low = ph+qh, det = ph-qh (contiguous tensor_tensor)
  All DMAs on the sync (HWDGE) ring.
"""

import numpy as np

import concourse.bacc as bacc
import concourse.mybir as mybir
import concourse.tile as tile
from concourse.bass_utils import run_bass_kernel_spmd

B, C, H, W = 16, 64, 512, 512
NCORES = 8
BPC = B // NCORES            # batches per core
P = BPC * C                  # 128 planes per core = SBUF partitions
R = 8                        # output rows per plane per iteration
ITERS = (H // 2) // R        # 32
F32 = mybir.dt.float32

TRACE = False                # test.py sets this for profiling runs
LAST_RESULTS = None          # BassKernelResults of the last run (for test.py)

_nc = None


def _build():
    nc = bacc.Bacc("TRN2", target_bir_lowering=False, debug=False)
    x = nc.dram_tensor("x", [P, H, W], F32, kind="ExternalInput")
    low = nc.dram_tensor("low", [P, H // 2, W // 2], F32, kind="ExternalOutput")
    det = nc.dram_tensor("det", [P, H // 2, W // 2], F32, kind="ExternalOutput")

    with tile.TileContext(nc) as tc:
        with (
            tc.tile_pool(name="inp", bufs=2) as inp,
            tc.tile_pool(name="pq", bufs=2) as pqp,
            tc.tile_pool(name="half", bufs=2) as hp,
            tc.tile_pool(name="outs", bufs=2) as op_,
        ):
            for i in range(ITERS):
                t = inp.tile([P, 2 * R, W], F32, tag="t")
                nc.sync.dma_start(out=t[:], in_=x[:, 2 * R * i:2 * R * (i + 1), :])
                a = t[:, 0:2 * R:2, 0:W:2]
                b = t[:, 0:2 * R:2, 1:W:2]
                c = t[:, 1:2 * R:2, 0:W:2]
                d = t[:, 1:2 * R:2, 1:W:2]
                p = pqp.tile([P, R, W // 2], F32, tag="p")
                q = pqp.tile([P, R, W // 2], F32, tag="q")
                nc.vector.tensor_tensor(out=p[:], in0=a, in1=d, op=mybir.AluOpType.add)
                nc.vector.tensor_tensor(out=q[:], in0=b, in1=c, op=mybir.AluOpType.add)
                ph = hp.tile([P, R, W // 2], F32, tag="ph")
                qh = hp.tile([P, R, W // 2], F32, tag="qh")
                nc.scalar.mul(out=ph[:], in_=p[:], mul=0.5)
                nc.scalar.mul(out=qh[:], in_=q[:], mul=0.5)
                lo = op_.tile([P, R, W // 2], F32, tag="lo")
                de = op_.tile([P, R, W // 2], F32, tag="de")
                nc.gpsimd.tensor_tensor(out=lo[:], in0=ph[:], in1=qh[:],
                                        op=mybir.AluOpType.add)
                nc.gpsimd.tensor_tensor(out=de[:], in0=ph[:], in1=qh[:],
                                        op=mybir.AluOpType.subtract)
                nc.sync.dma_start(out=low[:, R * i:R * (i + 1), :], in_=lo[:])
                nc.sync.dma_start(out=det[:, R * i:R * (i + 1), :], in_=de[:])
    nc.compile()
    return nc


def _get_nc():
    global _nc
    if _nc is None:
        _nc = _build()
    return _nc


def kernel(x):
    global LAST_RESULTS
    x = np.ascontiguousarray(np.asarray(x), dtype=np.float32)
    assert x.shape == (B, C, H, W), x.shape
    nc = _get_nc()
    in_maps = [
        {"x": x[i * BPC:(i + 1) * BPC].reshape(P, H, W)} for i in range(NCORES)
    ]
    last_err = None
    for _attempt in range(3):
        try:
            res = run_bass_kernel_spmd(nc, in_maps, list(range(NCORES)),
                                       trace=TRACE)
            break
        except Exception as e:  # transient NRT device errors happen; retry
            last_err = e
    else:
        raise last_err
    LAST_RESULTS = res
    low = np.concatenate(
        [r["low"].reshape(BPC, C, H // 2, W // 2) for r in res.results], axis=0)
    det = np.concatenate(
        [r["det"].reshape(BPC, C, H // 2, W // 2) for r in res.results], axis=0)
    return (low, det)
